# revision 28
# baseline (speedup 1.0000x reference)
"""Trainium2 Bass kernel for nn_NeuralGPKernel (sparse_attention).

Self-contained: hardcodes all shapes. Shards (B=2) x (N_q in 4 chunks of 128)
across 8 NeuronCores; each core computes mean/var for its 128 queries.

Math restructuring vs the reference:
  - ki @ kw1 decomposes: ki = [pos_q, pos_o, pos_q - pos_o], so
    hidden[q,o,:] = u[q,:] + w[o,:] with u = pos_q @ (A+C), w = pos_o @ (B-C) + kb1
    where kw1 = [A; B; C] (3 blocks of 3 rows).
  - softmax row-constants dropped: kb2 and |pos_q|^2 terms cancel in softmax.
  - log(rbf + 1e-8) ~= -dist2 / (sigma^2 + 1e-6)  (error <= 2e-7 for dist2<=3).
  - weighted variance = E[v^2] - E[v]^2 (weights sum to 1).
Per query q: logits^T[(q,h), o] = sum_k relu(w[o,k]+u[q,k]) kw2[k,h]
             + (-1/s2f[h]) * (|pos_o|^2 - 2 pos_q . pos_o)
computed via block-diagonal kw2 matmuls accumulating 16 queries x 8 heads
into one PSUM bank [128, 512].
"""

import sys
import types
import numpy as np

B, N_O, N_Q = 2, 512, 512
POS_DIM, LATENT, HEADS, HEAD_DIM, OUT_DIM = 3, 256, 8, 32, 128
HD = HEADS * HEAD_DIM
N_CORES = 8
NQ_C = N_Q * B // N_CORES  # 128 queries per core
QB = 16                     # queries per logits block
NBLK = NQ_C // QB           # 8 blocks per core

LAST_RESULT = None  # test.py reads exec_time_ns from here
RELU_PATTERN = "DDADDA"  # relu engine split: D=vector, A=scalar(ACT)


def _install_ntff_hook():
    """bass_utils wants antenv.axon_hooks for trace=True; provide it."""
    if "antenv.axon_hooks" in sys.modules:
        return
    try:
        import trn_agent_boot.trn_boot as tb
        hook = tb._ntff_profile_via_ctypes("/opt/axon/libaxon_pjrt.so")
    except Exception:
        hook = None
    m = types.ModuleType("antenv.axon_hooks")
    m.get_axon_ntff_profile_hook = lambda: hook
    m.set_axon_ntff_profile_hook = lambda h: None
    sys.modules["antenv.axon_hooks"] = m


def build_program():
    import concourse.bass as bass
    import concourse.mybir as mybir
    import concourse.tile as tile
    from concourse import bacc
    from concourse.masks import make_identity
    from contextlib import ExitStack

    f32 = mybir.dt.float32
    bf16 = mybir.dt.bfloat16
    AX = mybir.AxisListType
    ALU = mybir.AluOpType
    AF = mybir.ActivationFunctionType

    nc = bacc.Bacc("TRN2", target_bir_lowering=False, debug=False)

    def din(name, shape):
        return nc.dram_tensor(name, shape, f32, kind="ExternalInput").ap()

    def dout(name, shape):
        return nc.dram_tensor(name, shape, f32, kind="ExternalOutput").ap()

    h_obs = din("h_obs", [N_O, LATENT])
    pos_obs = din("pos_obs", [N_O, POS_DIM])
    pos_q = din("pos_q", [NQ_C, POS_DIM])
    pos_obs_T = din("pos_obs_T", [POS_DIM, N_O])
    pos_q_T = din("pos_q_T", [POS_DIM, NQ_C])
    fw1 = din("fw1", [LATENT, LATENT])
    fb1 = din("fb1", [LATENT])
    fw2 = din("fw2", [LATENT, HD])
    fb2 = din("fb2", [HD])
    log_sigma = din("log_sigma", [HEADS])
    kw1 = din("kw1", [POS_DIM * 3, LATENT])
    kb1 = din("kb1", [LATENT])
    kw2 = din("kw2", [LATENT, HEADS])
    ow = din("ow", [HD, OUT_DIM])
    ob = din("ob", [OUT_DIM])
    vw = din("vw", [HD, OUT_DIM])
    vb = din("vb", [OUT_DIM])
    mean_o = dout("mean", [NQ_C, OUT_DIM])
    var_o = dout("var", [NQ_C, OUT_DIM])

    def ap(t, offset, dims):
        return bass.AP(tensor=t.tensor, offset=t.offset + offset, ap=list(dims))

    with tile.TileContext(nc) as tc:
        st = ExitStack()
        _keep = []

        def T(shape, name, dt=f32):
            t, free = tc.tile(shape, dt, name=name)
            _keep.append(free)
            return t

        # ---------------- persistent SBUF tiles ----------------
        ident = T([128, 128], "ident")
        make_identity(nc, ident[:])
        ident_bf = T([128, 128], "ident_bf", bf16)
        nc.vector.tensor_copy(out=ident_bf[:], in_=ident[:])

        ls_rep = T([128, 1], "ls_rep")       # log_sigma[h] at partition (q*8+h)
        s2inv_neg = T([128, 1], "s2inv_neg")  # -1/(exp(2 ls)+1e-6), same layout
        s2rep4 = T([4, 128], "s2rep4")  # -1/s2f[c%8] replicated on 4 partitions
        w_bd = [T([128, 128], f"w_bd{k}", bf16) for k in range(2)]  # col-tiled blockdiag kw2
        kw2_sb = [T([128, HEADS], f"kw2s{k}") for k in range(2)]
        a3 = T([3, LATENT], "a3")
        b3 = T([3, LATENT], "b3")
        c3 = T([3, LATENT], "c3")
        AC = T([3, LATENT], "AC")
        BC = T([3, LATENT], "BC")
        lhsT_df = T([3, NQ_C], "lhsT_df")
        pq4 = T([4, NQ_C], "pq4")  # rows 0-2 pos_q^T, row 3 ones (f32)
        pos_oT = T([3, N_O], "pos_oT")
        rhs_d = T([4, N_O], "rhs_d", bf16)   # rows 0-2: -2 pos_o^T, row 3: |pos_o|^2
        sq3 = T([3, N_O], "sq3")
        ones3 = T([3, 1], "ones3")
        tmp_po2 = T([1, N_O], "tmp_po2", bf16)
        ones1_bf = T([1, 128], "ones1_bf", bf16)
        ones1 = T([1, 128], "ones1")
        u_kt = [T([128, NQ_C], f"u_kt{k}") for k in range(2)]
        wT = [T([128, N_O], f"wT{k}", bf16) for k in range(2)]
        kb1_col = [T([128, 1], f"kb1c{k}") for k in range(2)]
        fb1_col = [T([128, 1], f"fb1c{k}") for k in range(2)]
        fb2_row = T([1, HD], "fb2_row")
        ob_row = T([1, OUT_DIM], "ob_row")
        vb_row = T([1, OUT_DIM], "vb_row")
        fw1_sb = [T([128, LATENT], f"fw1s{k}") for k in range(2)]
        fw2_sb = [T([128, HD], f"fw2s{k}") for k in range(2)]
        ow_sb = [T([128, OUT_DIM], f"ows{k}") for k in range(2)]
        vw_sb = [T([128, OUT_DIM], f"vws{k}") for k in range(2)]
        ho = [T([128, LATENT], f"ho{k}") for k in range(4)]
        hT = [T([128, N_O], f"hT{k}") for k in range(2)]
        hidT = [T([128, N_O], f"hidT{k}") for k in range(2)]
        v_sb = [T([128, HD], f"v{k}", bf16) for k in range(4)]
        v2_sb = [T([128, HD], f"v2{k}", bf16) for k in range(4)]
        WT = [T([128, NQ_C * HEADS], f"WT{k}", bf16) for k in range(4)]
        hqT = [T([128, NQ_C], f"hqT{k}") for k in range(2)]
        sqT = [T([128, NQ_C], f"sqT{k}") for k in range(2)]
        varT = [T([128, NQ_C], f"varT{k}") for k in range(2)]
        mean_sb = T([NQ_C, OUT_DIM], "mean_sb")
        var_sb = T([NQ_C, OUT_DIM], "var_sb")

        # ---------------- small-input preprocessing ----------------
        # Critical-path gathers first (sync queue): pos transposes feed the
        # u/w matmuls which gate the entire block phase.
        nc.sync.dma_start(out=pos_oT[:], in_=pos_obs_T[:])
        nc.sync.dma_start(out=lhsT_df[:], in_=pos_q_T[:])
        nc.sync.dma_start(out=a3[:], in_=kw1[0:3, :])
        nc.sync.dma_start(out=b3[:], in_=kw1[3:6, :])
        nc.sync.dma_start(out=c3[:], in_=kw1[6:9, :])
        for k in range(2):
            nc.sync.dma_start(out=kb1_col[k][:], in_=kb1[128 * k : 128 * (k + 1)])
            nc.sync.dma_start(out=kw2_sb[k][:], in_=kw2[128 * k : 128 * (k + 1), :])
        nc.vector.tensor_add(AC[:], a3[:], c3[:])
        nc.vector.tensor_sub(BC[:], b3[:], c3[:])
        nc.vector.memset(ones3[:], 1.0)
        nc.vector.memset(ones1[:], 1.0)
        nc.vector.memset(ones1_bf[:], 1.0)
        nc.vector.tensor_mul(sq3[:], pos_oT[:], pos_oT[:])
        nc.scalar.mul(out=rhs_d[0:3, :], in_=pos_oT[:], mul=-2.0)

        # remaining input loads (DMA engines run these in parallel)
        for k in range(2):
            nc.sync.dma_start(out=fb1_col[k][:], in_=fb1[128 * k : 128 * (k + 1)])
            nc.sync.dma_start(out=fw1_sb[k][:], in_=fw1[128 * k : 128 * (k + 1), :])
            nc.sync.dma_start(out=fw2_sb[k][:], in_=fw2[128 * k : 128 * (k + 1), :])
            nc.sync.dma_start(out=ow_sb[k][:], in_=ow[128 * k : 128 * (k + 1), :])
            nc.sync.dma_start(out=vw_sb[k][:], in_=vw[128 * k : 128 * (k + 1), :])
        nc.sync.dma_start(out=fb2_row[:], in_=fb2[:])
        nc.sync.dma_start(out=ob_row[:], in_=ob[:])
        nc.sync.dma_start(out=vb_row[:], in_=vb[:])
        for k in range(4):
            nc.sync.dma_start(out=ho[k][:], in_=h_obs[128 * k : 128 * (k + 1), :])

        # sigma / rank-1 / block-diag weight builds (needed from block 0 on)
        nc.gpsimd.dma_start(out=ls_rep[:], in_=ap(log_sigma, 0, [[0, 16], [1, 8]]))
        nc.scalar.activation(out=s2inv_neg[:], in_=ls_rep[:], func=AF.Exp, scale=2.0)
        nc.vector.tensor_scalar_add(out=s2inv_neg[:], in0=s2inv_neg[:], scalar1=1e-6)
        nc.vector.reciprocal(out=s2inv_neg[:], in_=s2inv_neg[:])
        nc.scalar.mul(out=s2inv_neg[:], in_=s2inv_neg[:], mul=-1.0)
        for r in range(4):
            nc.gpsimd.dma_start(out=s2rep4[r : r + 1, :], in_=s2inv_neg[:, 0:1])
        nc.gpsimd.dma_start(out=pq4[0:3, :], in_=pos_q_T[:])
        nc.gpsimd.dma_start(out=pq4[3:4, :], in_=ones1[0:1, :])
        # col-tiled block-diagonal kw2: variant jj lives at cols [32jj,32jj+32),
        # nonzero at local cols [8jj, 8jj+8)
        for k in range(2):
            nc.vector.memset(w_bd[k][:], 0.0)
            for jj in range(4):
                nc.vector.tensor_copy(
                    out=w_bd[k][:, 40 * jj : 40 * jj + 8], in_=kw2_sb[k][:]
                )

        st0 = st.enter_context(ExitStack())
        pp_a = st0.enter_context(tc.tile_pool(name="pp_a", bufs=2, space="PSUM"))
        pp_b = st0.enter_context(tc.tile_pool(name="pp_b", bufs=2, space="PSUM"))

        # |pos_o|^2 row of rhs_d
        ps1 = pp_b.tile([1, N_O], f32, tag="pp_b", name="ps1")
        nc.tensor.matmul(ps1[:], lhsT=ones3[:], rhs=sq3[:], start=True, stop=True)
        nc.vector.tensor_copy(out=tmp_po2[:], in_=ps1[:])
        nc.gpsimd.dma_start(out=rhs_d[3:4, :], in_=tmp_po2[:])

        # u^T and w^T (kernel-MLP rank factors)
        for mt in range(2):
            psu = pp_b.tile([128, NQ_C], f32, tag="pp_b", name="psu")
            nc.tensor.matmul(
                psu[:], lhsT=AC[:, 128 * mt : 128 * (mt + 1)], rhs=lhsT_df[:],
                start=True, stop=True,
            )
            nc.vector.tensor_copy(out=u_kt[mt][:], in_=psu[:])
            psw = pp_a.tile([128, N_O], f32, tag="pp_a", name="psw")
            nc.tensor.matmul(
                psw[:], lhsT=BC[:, 128 * mt : 128 * (mt + 1)], rhs=pos_oT[:],
                start=True, stop=True,
            )
            nc.scalar.activation(
                out=wT[mt][:], in_=psw[:], func=AF.Identity, bias=kb1_col[mt][:]
            )

        # ---------------- per-block logits + softmax + transpose ----------------
        st0.close()
        spool = st.enter_context(tc.tile_pool(name="spool", bufs=12))
        wpool = st.enter_context(tc.tile_pool(name="wpool", bufs=6))
        pp_t = st.enter_context(tc.tile_pool(name="pp_t", bufs=2, space="PSUM"))
        pp_att = st.enter_context(tc.tile_pool(name="pp_att", bufs=1, space="PSUM"))
        st1 = st.enter_context(ExitStack())
        pp_l = st1.enter_context(tc.tile_pool(name="pp_l", bufs=2, space="PSUM"))

        def emit_produce(i):
            lps = pp_l.tile([128, N_O], f32, tag="logits", name="lps")
            ridx = i * 32
            for kt in range(2):
                for wave in range(4):
                    stiles = []
                    for g in range(4):
                        j = 4 * g + wave
                        qg = QB * i + j
                        s_t = spool.tile([128, N_O], bf16, tag="s", name="s_t")
                        ucol = u_kt[kt][:, qg : qg + 1]
                        eng = RELU_PATTERN[ridx % len(RELU_PATTERN)]
                        ridx += 1
                        if eng == "D":
                            nc.vector.tensor_scalar(
                                out=s_t[:], in0=wT[kt][:], scalar1=ucol, scalar2=0.0,
                                op0=ALU.add, op1=ALU.max,
                            )
                        else:
                            nc.scalar.activation(
                                out=s_t[:], in_=wT[kt][:], func=AF.Relu, bias=ucol
                            )
                        stiles.append((g, s_t))
                    for g, s_t in stiles:
                        nc.tensor.matmul(
                            lps[32 * g : 32 * g + 32, :],
                            lhsT=w_bd[kt][:, 32 * wave : 32 * (wave + 1)], rhs=s_t[:],
                            start=(kt == 0 and wave == 0), stop=False,
                            tile_position=(0, 32 * g), skip_group_check=True,
                        )
            # dist2 term fused as one rank-4 matmul: lhsT col (8j+h) rows:
            # 0-2 = pos_q[16i+j,p] * (-1/s2f[h]), 3 = -1/s2f[h]; rhs rows:
            # 0-2 = -2 pos_o^T, 3 = |pos_o|^2
            dh = wpool.tile([4, 128], bf16, tag="dh", name="dh")
            _o = dh[:]
            _p = pq4[:]
            _s = s2rep4[:]
            nc.vector.tensor_tensor(
                out=ap(_o, 0, [_o.ap[0], [8, QB], [1, 8]]),
                in0=ap(_p, QB * i, [_p.ap[0], [1, QB], [0, 8]]),
                in1=ap(_s, 0, [_s.ap[0], [8, QB], [1, 8]]),
                op=ALU.mult,
            )
            nc.tensor.matmul(lps[:], lhsT=dh[:], rhs=rhs_d[:], start=False, stop=True,
                             skip_group_check=True)
            return lps

        def emit_softmax(i, lps):
            # softmax over o (free dim), normalized in place
            sums = wpool.tile([128, 1], f32, tag="sums", name="sums")
            recip = wpool.tile([128, 1], f32, tag="recip", name="recip")
            W_t = wpool.tile([128, N_O], bf16, tag="W", name="W_t")
            nc.scalar.activation(
                out=W_t[:], in_=lps[:], func=AF.Exp, accum_out=sums[:]
            )
            nc.vector.reciprocal(out=recip[:], in_=sums[:])
            nc.vector.tensor_scalar_mul(out=W_t[:], in0=W_t[:], scalar1=recip[:])

            # transpose W [128qh, 512o] -> WT[ot][:, i*128...]
            for ot in range(4):
                pst = pp_t.tile([128, 128], bf16, tag="t", name="pstw")
                nc.tensor.transpose(
                    pst[:], in_=W_t[:, 128 * ot : 128 * (ot + 1)], identity=ident_bf[:]
                )
                nc.vector.tensor_copy(
                    out=WT[ot][:, 128 * i : 128 * (i + 1)], in_=pst[:]
                )

        # feature net v = relu(h fw1 + fb1) fw2 + fb2, emitted in chunks
        # interleaved with the block loop so in-order engine queues stay fed
        def feat_transposes():
            for ot in range(4):
                for ct in range(2):
                    pst = pp_t.tile([128, 128], f32, tag="t", name="pstf")
                    nc.tensor.transpose(
                        pst[:], in_=ho[ot][:, 128 * ct : 128 * (ct + 1)],
                        identity=ident[:],
                    )
                    nc.vector.tensor_copy(
                        out=hT[ct][:, 128 * ot : 128 * (ot + 1)], in_=pst[:]
                    )

        def feat_hidden(mt):
            def f():
                psh = pp_t.tile([128, N_O], f32, tag="t", name="psh")
                for kt in range(2):
                    nc.tensor.matmul(
                        psh[:], lhsT=fw1_sb[kt][:, 128 * mt : 128 * (mt + 1)],
                        rhs=hT[kt][:], start=(kt == 0), stop=(kt == 1),
                    )
                nc.scalar.activation(
                    out=hidT[mt][:], in_=psh[:], func=AF.Relu, bias=fb1_col[mt][:]
                )
            return f

        def feat_v(ots):
            def f():
                for ot in ots:
                    psv = pp_t.tile([128, HD], f32, tag="t", name="psv")
                    for kt in range(2):
                        nc.tensor.matmul(
                            psv[:], lhsT=hidT[kt][:, 128 * ot : 128 * (ot + 1)],
                            rhs=fw2_sb[kt][:], start=(kt == 0), stop=False,
                        )
                    nc.tensor.matmul(psv[:], lhsT=ones1[:], rhs=fb2_row[:],
                                     start=False, stop=True)
                    nc.vector.tensor_copy(out=v_sb[ot][:], in_=psv[:])
                    nc.vector.tensor_mul(v2_sb[ot][:], v_sb[ot][:], v_sb[ot][:])
            return f

        pm = [pp_att.tile([128, NQ_C], f32, tag=f"pm{k}", name=f"pm{k}") for k in range(2)]
        pe = [pp_att.tile([128, NQ_C], f32, tag=f"pe{k}", name=f"pe{k}") for k in range(2)]

        def emit_attention(half):
            q0 = 64 * half
            for h in range(HEADS):
                k = h // 4
                r0 = 32 * (h % 4)
                for ot in range(4):
                    wt_h = WT[ot][:].rearrange("p (q h) -> p h q", h=HEADS)[
                        :, h, q0 : q0 + 64
                    ]
                    nc.tensor.matmul(
                        pm[k][r0 : r0 + 32, q0 : q0 + 64],
                        lhsT=v_sb[ot][:, 32 * h : 32 * (h + 1)], rhs=wt_h,
                        start=(ot == 0), stop=(ot == 3), tile_position=(0, r0),
                    )
                    nc.tensor.matmul(
                        pe[k][r0 : r0 + 32, q0 : q0 + 64],
                        lhsT=v2_sb[ot][:, 32 * h : 32 * (h + 1)], rhs=wt_h,
                        start=(ot == 0), stop=(ot == 3), tile_position=(0, r0),
                    )

        chunks = [feat_transposes, feat_hidden(0), feat_hidden(1),
                  feat_v([0, 1]), feat_v([2, 3])]
        prev = None
        for i in range(NBLK):
            lps = emit_produce(i)
            if prev is not None:
                emit_softmax(i - 1, prev)
            if chunks:
                chunks.pop(0)()
            if i == 5:
                emit_attention(0)
            prev = lps
        emit_softmax(NBLK - 1, prev)
        emit_attention(1)

        # ---------------- attention epilogue ----------------
        st1.close()
        for k in range(2):
            nc.vector.tensor_copy(out=hqT[k][:], in_=pm[k][:])
            nc.vector.tensor_mul(sqT[k][:], hqT[k][:], hqT[k][:])
            nc.vector.tensor_sub(varT[k][:], pe[k][:], sqT[k][:])

        # ---------------- output projections ----------------
        pso = pp_t.tile([NQ_C, OUT_DIM], f32, tag="t", name="pso")
        for k in range(2):
            nc.tensor.matmul(pso[:], lhsT=hqT[k][:], rhs=ow_sb[k][:],
                             start=(k == 0), stop=False)
        nc.tensor.matmul(pso[:], lhsT=ones1[:], rhs=ob_row[:], start=False, stop=True)
        nc.vector.tensor_copy(out=mean_sb[:], in_=pso[:])
        nc.sync.dma_start(out=mean_o[:], in_=mean_sb[:])

        psv2 = pp_t.tile([NQ_C, OUT_DIM], f32, tag="t", name="psv2")
        for k in range(2):
            nc.tensor.matmul(psv2[:], lhsT=varT[k][:], rhs=vw_sb[k][:],
                             start=(k == 0), stop=False)
        nc.tensor.matmul(psv2[:], lhsT=ones1[:], rhs=vb_row[:], start=False, stop=True)
        # softplus(x) = ln(1 + exp(x))
        nc.scalar.activation(out=var_sb[:], in_=psv2[:], func=AF.Exp)
        nc.vector.tensor_scalar_add(out=var_sb[:], in0=var_sb[:], scalar1=1.0)
        nc.scalar.activation(out=var_sb[:], in_=var_sb[:], func=AF.Ln)
        nc.sync.dma_start(out=var_o[:], in_=var_sb[:])

        st.close()
        for f in reversed(_keep):
            f()

    nc.compile()
    return nc


_NC = None


def _get_nc():
    global _NC
    if _NC is None:
        _NC = build_program()
    return _NC


def shard_inputs(inputs):
    """Build per-core input maps from full inputs."""
    g = {k: np.ascontiguousarray(np.asarray(v, dtype=np.float32)) for k, v in inputs.items()}
    maps = []
    for c in range(N_CORES):
        b, qi = c // 4, c % 4
        maps.append({
            "h_obs": g["h_obs"][b],
            "pos_obs": g["pos_obs"][b],
            "pos_q": np.ascontiguousarray(g["pos_query"][b, 128 * qi : 128 * (qi + 1)]),
            "pos_obs_T": np.ascontiguousarray(g["pos_obs"][b].T),
            "pos_q_T": np.ascontiguousarray(g["pos_query"][b, 128 * qi : 128 * (qi + 1)].T),
            "fw1": g["fw1"], "fb1": g["fb1"], "fw2": g["fw2"], "fb2": g["fb2"],
            "log_sigma": g["log_sigma"],
            "kw1": g["kw1"], "kb1": g["kb1"], "kw2": g["kw2"],
            "ow": g["ow"], "ob": g["ob"], "vw": g["vw"], "vb": g["vb"],
        })
    return maps


def kernel(**inputs):
    global LAST_RESULT
    _install_ntff_hook()
    from concourse.bass_utils import run_bass_kernel_spmd
    import os

    nc = _get_nc()
    maps = shard_inputs(inputs)
    trace = bool(int(os.environ.get("KERNEL_TRACE", "0")))
    res = run_bass_kernel_spmd(nc, maps, list(range(N_CORES)), trace=trace)
    LAST_RESULT = res
    mean = np.zeros((B, N_Q, OUT_DIM), np.float32)
    var = np.zeros((B, N_Q, OUT_DIM), np.float32)
    for c in range(N_CORES):
        b, qi = c // 4, c % 4
        mean[b, 128 * qi : 128 * (qi + 1)] = res.results[c]["mean"]
        var[b, 128 * qi : 128 * (qi + 1)] = res.results[c]["var"]
    return (mean, var)
